# revision 37
# baseline (speedup 1.0000x reference)
"""Trainium2 Bass kernel for nn_Encoder_83992380441041 (causal linear attention
encoder, last-position readout).

Math (per segment b of T tokens):
    yn   = LayerNorm(x_b) * gamma + beta          (beta == 0 here)
    K    = phi(yn @ Wk.T); V = yn @ Wv.T; q = phi(yn[T-1] @ Wq.T)
    out  = q @ (K.T V) / (q . sum_t K_t + eps)    [only last position matters]
with phi(a) = elu(a)+1 = min(exp(a),1) + relu(a).

Key folds:
  * gamma into the weights (host).
  * centering into the weights (host): x @ (W - 1 s~/d) = (x - mu 1) @ W.
  * 1/sqrt(var+eps) via ACT ln/exp (r = exp(-0.5 ln(var128/128+eps))) so the
    ACT table (natural_log_exp) is loaded exactly once; no Sqrt table switch.
  * r applied to RAW x in [token, d] layout (r is per-partition there), an
    SBUF->SBUF op gpsimd can run; everything downstream is pre-normalized.
  * bf16 for the transposed x, the weights and K/V (PE at 1 cyc/row).
  * gpsimd does all SBUF-side elementwise (it cannot touch PSUM); DVE/ACT
    split the PSUM->SBUF extraction.

Pipeline: 5 DMA blocks of [2,4,4,4,2] tiles; per block:
  bn_stats (DVE) -> var combine (gpsimd) -> r (ACT ln/exp) -> xs = r*x
  (gpsimd) -> transposes (PE) -> xct bf16 (DVE); G matmuls (PE bf16) ->
  K copy (DVE) | V copy (ACT) -> phi exp (ACT) + relu/combine (gpsimd)
  -> S|Z matmuls (PE bf16, 2 segs column-packed).
Readout: q from xct columns (already normalized), S.q via PE.

Sharding: data-parallel over segments. 64 segments -> 8 cores x 8 segments.
"""

import numpy as np

import concourse.bass as bass
import concourse.tile as tile
from concourse import mybir
from concourse.bass_utils import run_bass_kernel_spmd
from concourse.vector_clock import ScopedClock
import bass_rust

EPS_LN = 1e-5
EPS_DEN = 1e-5

F32 = mybir.dt.float32
BF16 = mybir.dt.bfloat16
F32R = mybir.dt.float32r
AF = mybir.ActivationFunctionType
ALU = mybir.AluOpType

N_CORES = 8

import os as _os
# t = f32r transposes (1.5 cyc/row vs 2.0 for f32)
_F32R_MODE = _os.environ.get("KERNEL_F32R", "")


def _r(ap, on):
    return ap.bitcast(F32R) if on else ap


def _patched_drain_and_barrier(self, tick_clock, wait_clock):
    # Stock TileContext exit puts one sem-wait per outstanding proc on a
    # single InstDrain; walrus in this container caps sync waits per
    # instruction. Split them across a chain of drains on the same engine
    # (program order preserved => equivalent).
    nc = self.nc
    drain_inst = nc.sync.drain()
    wait_clock.add_sem_waits(
        drain_inst.ins, ScopedClock({None: tick_clock.global_clock})
    )
    si = drain_inst.ins.sync_info
    if si is not None and si.on_wait is not None and len(si.on_wait) > 1:
        waits = list(si.on_wait)
        si.on_wait = waits[:1]
        engines = [nc.sync, nc.scalar, nc.vector, nc.gpsimd, nc.tensor]
        for i, w in enumerate(waits[1:]):
            d2 = engines[(i + 1) % len(engines)].drain()
            si2 = d2.ins.sync_info
            if si2 is None:
                d2.ins.sync_info = bass_rust.SyncInfo(on_wait=[w], on_update=[])
            else:
                si2.on_wait = [w]
    nc.all_engine_barrier()
    assert self.sems is not None
    popped = nc._tile_sem_poison_stack.pop()
    assert popped is self._sem_poison
    nc.clear_and_free_semaphores(list(self.sems.allocated().values()))


tile.TileContext._drain_and_barrier = _patched_drain_and_barrier

_orig_commit = tile.TileContext._commit_instruction
_wsplit_counter = [0]


def _patched_commit_instruction(self, inst, lazy_reg_writes: bool = True):
    # Enforce the per-instruction sync-wait capacity of the walrus in this
    # container (1 for regular instructions, 2 for EventSemaphore) by
    # spilling excess waits onto same-engine NOPs committed just before.
    si = getattr(inst, "sync_info", None)
    if si is not None and si.on_wait:
        cap = 2 if isinstance(inst, mybir.InstEventSemaphore) else 1
        if len(si.on_wait) > cap:
            waits = list(si.on_wait)
            si.on_wait = waits[:cap]
            for w in waits[cap:]:
                _wsplit_counter[0] += 1
                nop = mybir.InstNoOp(
                    name=f"wsplit-{_wsplit_counter[0]}",
                    sync_info=mybir.SyncInfo(on_wait=[w], on_update=[]),
                    bass_nofuse=True,
                    engine=inst.engine,
                )
                _orig_commit(self, nop, lazy_reg_writes=False)
    return _orig_commit(self, inst, lazy_reg_writes=lazy_reg_writes)


tile.TileContext._commit_instruction = _patched_commit_instruction


# tiles per DMA/compute block: small first (earliest pipeline start) and
# small last (short tail chain after the final DMA byte lands)
BLOCKS = tuple(
    int(v) for v in _os.environ.get("KERNEL_BLOCKS", "2,4,4,4,2").split(",")
)
JUNK = tuple(
    int(v) for v in _os.environ.get("KERNEL_JUNK", "16,64,256").split(",") if v
)


def _build(n_tok: int, n_seg: int, d: int, f: int):
    """Per-core program. Inputs: x [n_tok,d]; wpack [128, 320] =
    [wkv~ | wq~ | ident] (weights f32, downcast on device).
    Output: z [n_seg,f]."""
    P = 128
    assert n_tok % P == 0 and d == P
    n_tiles = n_tok // P
    t_seg = n_tok // n_seg
    assert t_seg == 2 * P  # 2 tiles per segment
    f2 = 2 * f
    assert sum(BLOCKS) == n_tiles
    n_blk = len(BLOCKS)
    bounds = [0]
    for bb in BLOCKS:
        bounds.append(bounds[-1] + bb)

    nc = bass.Bass()
    XSENG = nc.vector if "v" in _os.environ.get("KERNEL_XS", "") else nc.gpsimd
    x_d = nc.declare_dram_parameter("x", [n_tok, d], F32, isOutput=False)
    wpack_d = nc.declare_dram_parameter("wpack", [P, f2 + f + P], F32,
                                        isOutput=False)
    z_d = nc.declare_dram_parameter("z", [n_seg, f], F32, isOutput=True)

    with tile.TileContext(nc) as tc:
        with (
            tc.tile_pool(name="singles", bufs=1) as singles,
            tc.tile_pool(name="phi", bufs=3) as phip,
            tc.tile_pool(name="sseg", bufs=1) as ssegp,
            tc.tile_pool(name="fin", bufs=1) as finp,
            tc.tile_pool(name="psT", bufs=2, space="PSUM") as psT,
            tc.tile_pool(name="psG", bufs=2, space="PSUM") as psG,
            tc.tile_pool(name="psS", bufs=2, space="PSUM") as psS,
            tc.tile_pool(name="psM", bufs=1, space="PSUM") as psM,
        ):
            # --- persistent buffers ---
            xbig = singles.tile([P, n_tok], F32)
            wpack = singles.tile([P, f2 + f + P], F32)
            wbf = singles.tile([P, f2 + f], BF16)
            identb = singles.tile([P, P], BF16)
            onecol = singles.tile([P, 1], BF16)
            xct = singles.tile([P, n_tok], BF16)
            kbig = singles.tile([P, n_tiles * f], BF16)
            vbig = singles.tile([P, n_tiles * f], BF16)
            bnb = singles.tile([P, n_tiles, 6], F32)
            mv = singles.tile([P, n_tiles, 2], F32)
            rbig = singles.tile([P, n_tiles], F32)
            eps_s = singles.tile([P, 1], F32)
            junk = singles.tile([P, 256], F32)
            lnj = singles.tile([1, 1], F32)

            # --- DMA triggers: x blocks staggered on sync (in consumption
            # order), wpack on scalar in parallel
            nc.scalar.dma_start(out=wpack[:], in_=wpack_d[:])
            xsrc = x_d.rearrange("(n p) d -> p n d", p=P)
            for b in range(n_blk):
                lo, hi = bounds[b], bounds[b + 1]
                nc.sync.dma_start(
                    out=xbig[:, lo * P:hi * P],
                    in_=xsrc[:, lo:hi, :],
                )

            # --- constants + ACT table preload (overlapped with DMA wait)
            nc.vector.memset(junk[:], 1.0)
            nc.vector.memset(eps_s[:], EPS_LN)
            nc.vector.memset(onecol[:], 1.0)
            # loads the natural_log_exp table once; Ln+Exp+Copy stay resident
            nc.scalar.activation(out=lnj[:], in_=junk[0:1, 0:1], func=AF.Ln)

            # PE warm-up: keep the HAM clock gate fed while DMAs are in
            # flight so the real stream runs at speed.
            pm = psM.tile([P, 512], F32, tag="m")
            for s in JUNK:
                nc.tensor.matmul(
                    pm[0:1, 0:s], lhsT=junk[:, 0:1], rhs=junk[:, 0:s],
                    start=True, stop=True, skip_group_check=True,
                )

            wkv_bf = wbf[:, 0:f2]
            wq_bf = wbf[:, f2:f2 + f]
            ident = wpack[:, f2 + f:f2 + f + P]
            nc.vector.tensor_copy(out=wbf[:], in_=wpack[:, 0:f2 + f])
            nc.vector.tensor_copy(out=identb[:], in_=ident)

            xview = xbig[:].rearrange("p (n d) -> p n d", d=P)

            pend = [None] * n_blk   # (e_t, s_t) per block
            s_sbs = [None] * n_blk

            def emit_relu(b):
                lo, hi = bounds[b], bounds[b + 1]
                gK, e_t, s_t = pend[b]
                nc.scalar.activation(out=s_t[:], in_=gK, func=AF.Relu)

            def emit_stt(b):
                lo, hi = bounds[b], bounds[b + 1]
                gK, e_t, s_t = pend[b]
                nc.vector.tensor_scalar_min(
                    out=e_t[:], in0=e_t[:], scalar1=1.0
                )
                nc.gpsimd.tensor_tensor(
                    out=kbig[:, lo * f:hi * f], in0=e_t[:], in1=s_t[:],
                    op=ALU.add,
                )

            def emit_S(b):
                lo, hi = bounds[b], bounds[b + 1]
                segs = list(range(lo // 2, hi // 2))
                s_ps = psS.tile([P, f + 1], F32, name="s_ps", tag="s")
                for s in segs:
                    hh = s % 2
                    for j in range(2):
                        n = 2 * s + j
                        nc.tensor.matmul(
                            s_ps[hh * f:(hh + 1) * f, 0:f],
                            lhsT=kbig[:, n * f:(n + 1) * f],
                            rhs=vbig[:, n * f:(n + 1) * f],
                            start=(j == 0), stop=(j == 1),
                            skip_group_check=True,
                            tile_position=(0, hh * f),
                        )
                    for j in range(2):
                        n = 2 * s + j
                        nc.tensor.matmul(
                            s_ps[hh * f:(hh + 1) * f, f:f + 1],
                            lhsT=kbig[:, n * f:(n + 1) * f],
                            rhs=onecol[:],
                            start=(j == 0), stop=(j == 1),
                            skip_group_check=True,
                            tile_position=(0, hh * f),
                        )
                s_sb = ssegp.tile([P, f + 1], F32, name="s_sb", tag=f"sb{b}")
                if len(segs) == 1:
                    hh = segs[0] % 2
                    rows = slice(hh * f, (hh + 1) * f)
                else:
                    rows = slice(0, P)
                nc.scalar.copy(out=s_sb[rows, :], in_=s_ps[rows, :])
                s_sbs[b] = (s_sb, rows)

            for b in range(n_blk):
                lo, hi = bounds[b], bounds[b + 1]
                nb = hi - lo

                # prev block: relu (ACT, reads PSUM) first so its phi chain
                # finishes; then per-tile stats on raw x
                if b > 0:
                    emit_relu(b - 1)
                for j in range(nb):
                    nc.vector.bn_stats(
                        out=bnb[:, lo + j, :], in_=xview[:, lo + j, :]
                    )
                    nc.vector.bn_aggr(
                        out=mv[:, lo + j, :], in_=bnb[:, lo + j, :]
                    )
                # r = exp(-0.5 ln(var + eps)) on ACT (table resident)
                nc.scalar.activation(
                    out=rbig[:, lo:hi], in_=mv[:, lo:hi, 1],
                    func=AF.Ln, bias=eps_s[:], scale=1.0,
                )
                nc.scalar.activation(
                    out=rbig[:, lo:hi], in_=rbig[:, lo:hi],
                    func=AF.Exp, scale=-0.5,
                )

                # xs = r * x (bf16) in raw [token, d] layout: ONE gpsimd op
                # per block via a stride-0 broadcast of r over features
                xs = phip.tile([P, nb * P], BF16, tag="xs")
                xs3 = xs[:].rearrange("p (n d) -> p n d", d=P)
                r3 = rbig[:, lo:hi].unsqueeze(-1).broadcast_to((P, nb, P))
                nc.gpsimd.tensor_tensor(
                    out=xs3, in0=xview[:, lo:hi, :], in1=r3, op=ALU.mult,
                )
                # prev block's phi combine + S (stt emitted before S reads K)
                if b > 0:
                    emit_stt(b - 1)
                    emit_S(b - 1)

                # bf16 transposes of the normalized rows
                pT = psT.tile([P, nb * P], BF16, name="pT", tag="t")
                for j in range(nb):
                    nc.tensor.matmul(
                        pT[:, j * P:(j + 1) * P],
                        lhsT=xs[:, j * P:(j + 1) * P],
                        rhs=identb[:], is_transpose=True,
                        start=True, stop=True, skip_group_check=True,
                    )
                # PSUM -> SBUF (DVE, bf16)
                nc.vector.tensor_copy(out=xct[:, lo * P:hi * P], in_=pT[:])

                # G split into K / V halves so downstream APs are contiguous
                gT = psG.tile([P, nb * f2], F32, name="gT", tag="g")
                gK = gT[:, 0:nb * f]
                gV = gT[:, nb * f:nb * f2]
                for j in range(nb):
                    nc.tensor.matmul(
                        gK[:, j * f:(j + 1) * f],
                        lhsT=xct[:, (lo + j) * P:(lo + j + 1) * P],
                        rhs=wkv_bf[:, 0:f],
                        start=True, stop=True, skip_group_check=True,
                    )
                    nc.tensor.matmul(
                        gV[:, j * f:(j + 1) * f],
                        lhsT=xct[:, (lo + j) * P:(lo + j + 1) * P],
                        rhs=wkv_bf[:, f:f2],
                        start=True, stop=True, skip_group_check=True,
                    )

                # phi: e on ACT; V extraction on ACT; relu/stt deferred
                e_t = phip.tile([P, nb * f], BF16, tag="e")
                s_t = phip.tile([P, nb * f], BF16, tag="s")
                nc.scalar.activation(out=e_t[:], in_=gK, func=AF.Exp)
                nc.scalar.copy(out=vbig[:, lo * f:hi * f], in_=gV)
                pend[b] = (gK, e_t, s_t)
                if b == n_blk - 1:
                    emit_relu(b)
                    emit_stt(b)
                    emit_S(b)

            # --- readout: q from xct columns (already normalized) ---
            qstack = finp.tile([P, n_seg], F32)
            ndsb = finp.tile([f + 1, n_seg], F32)
            zden = finp.tile([n_seg, 1], F32)
            zout = finp.tile([n_seg, f], F32)
            eq = finp.tile([P, n_seg], F32)
            sq8 = finp.tile([P, n_seg], F32)
            q2big = finp.tile([P, n_seg], F32)

            xq = xct[:, t_seg - 1::t_seg]
            qc_ps = pm[:, 272:272 + n_seg]
            nc.tensor.matmul(
                qc_ps[0:f, :], lhsT=wq_bf, rhs=xq,
                start=True, stop=True, skip_group_check=True,
            )
            nc.tensor.matmul(
                qc_ps[f:2 * f, :], lhsT=wq_bf, rhs=xq,
                start=True, stop=True, skip_group_check=True,
                tile_position=(0, f),
            )
            nc.scalar.activation(out=eq[:], in_=qc_ps, func=AF.Exp)
            nc.vector.tensor_scalar_max(out=sq8[:], in0=qc_ps, scalar1=0.0)
            nc.vector.scalar_tensor_tensor(
                out=q2big[:], in0=eq[:], scalar=1.0, in1=sq8[:],
                op0=ALU.min, op1=ALU.add,
            )
            # qstack col s: q on the (s%2) partition half, zero on the other
            nc.vector.memset(qstack[:], 0.0)
            nc.vector.tensor_copy(
                out=qstack[0:f, 0:n_seg:2], in_=q2big[0:f, 0:n_seg:2]
            )
            nc.vector.tensor_copy(
                out=qstack[f:2 * f, 1:n_seg:2], in_=q2big[f:2 * f, 1:n_seg:2]
            )

            ndT = pm[0:f + 1, 288:288 + n_seg]
            for b in range(n_blk):
                s0, s1 = bounds[b] // 2, bounds[b + 1] // 2
                s_sb, rows = s_sbs[b]
                nc.tensor.matmul(
                    ndT[:, s0:s1], lhsT=s_sb[rows, :],
                    rhs=qstack[rows, s0:s1],
                    start=True, stop=True, skip_group_check=True,
                )

            nc.vector.tensor_copy(out=ndsb[:], in_=ndT)
            nd_ps = pm[0:n_seg, 304:304 + f + 1]
            nc.tensor.transpose(nd_ps, ndsb[:], ident[0:f + 1, 0:f + 1])
            nc.vector.tensor_scalar_add(
                out=zden[:], in0=nd_ps[:, f:f + 1], scalar1=EPS_DEN
            )
            nc.vector.reciprocal(out=zden[:], in_=zden[:])
            nc.vector.tensor_scalar_mul(
                out=zout[:], in0=nd_ps[:, 0:f], scalar1=zden[:]
            )
            nc.sync.dma_start(out=z_d[:], in_=zout[:])

    return nc


def _prep(inputs):
    x = np.ascontiguousarray(np.asarray(inputs["x"], dtype=np.float32))
    batch = np.asarray(inputs["batch"]).astype(np.int64)
    gamma = np.asarray(inputs["gamma"], dtype=np.float32)
    beta = np.asarray(inputs["beta"], dtype=np.float32)
    wk = np.asarray(inputs["Wk"], dtype=np.float32)
    wq = np.asarray(inputs["Wq"], dtype=np.float32)
    wv = np.asarray(inputs["Wv"], dtype=np.float32)
    n_batches = int(np.asarray(inputs["n_batches"]))

    n, d = x.shape
    f = wk.shape[0]
    t_seg = n // n_batches
    counts = np.bincount(batch, minlength=n_batches)
    if not (np.all(counts == t_seg) and np.all(np.diff(batch) >= 0)):
        raise NotImplementedError("kernel specialized for equal sorted segments")
    if np.any(beta != 0.0):
        raise NotImplementedError("kernel specialized for beta == 0")

    wkg = (wk * gamma[None, :]).astype(np.float64)
    wvg = (wv * gamma[None, :]).astype(np.float64)
    wqg = (wq * gamma[None, :]).astype(np.float64)
    wkv_t = np.concatenate([wkg, wvg], axis=0).T            # [d, 2f]
    wq_t = wqg.T                                            # [d, f]
    # fold the LN centering into the weights:
    #   x @ (W - 1 s~/d) = (x - mu 1) @ W   since 1 @ W = colsums(W)
    wkv_t = wkv_t - wkv_t.sum(axis=0, keepdims=True) / d
    wq_t = wq_t - wq_t.sum(axis=0, keepdims=True) / d
    ident = np.eye(128, dtype=np.float64)
    wpack = np.ascontiguousarray(
        np.concatenate([wkv_t, wq_t, ident], axis=1).astype(np.float32)
    )

    return x, wpack, n, d, f, n_batches, t_seg


def _run(inputs, trace=False):
    x, wpack, n, d, f, n_batches, t_seg = _prep(inputs)

    segs_per_core = n_batches // N_CORES
    tok_per_core = segs_per_core * t_seg
    nc = _build(tok_per_core, segs_per_core, d, f)

    in_maps = []
    for c in range(N_CORES):
        m = {
            "x": np.ascontiguousarray(x[c * tok_per_core:(c + 1) * tok_per_core]),
            "wpack": wpack,
        }
        in_maps.append(m)

    res = run_bass_kernel_spmd(nc, in_maps, list(range(N_CORES)), trace=trace)
    z = np.concatenate([res.results[c]["z"] for c in range(N_CORES)], axis=0)
    return z, res


def kernel(**inputs) -> np.ndarray:
    z, _ = _run(inputs, trace=False)
    return z
